# revision 34
# baseline (speedup 1.0000x reference)
"""BCH/RS systematic encoder kernel for Trainium2 (8 NeuronCores, data parallel).

Computes out = concat([msg, (msg @ Gp) mod 2], axis=-1) for
msg [16384, 1000] f32 of 0/1 bits and Gp [1000, 256] f32 of 0/1 bits.

Design v20 (per core, 2048 rows = 4 chunks of 512; raw bass, no Tile):
  - Host assembles the systematic half of the codeword from the input;
    the device only computes the parity block (removes the 10.3 MB/core
    f32 output write of the original design). Per-core HBM traffic:
    2.36 MB fp8 read + 1.05 MB i16 write.
  - Host ships msg as fp8e4 (exact 0/1) pre-transposed to the matmul
    moving layout msgt[q, c, g, i, m] = msg[512c + m, 256g + 128i + q];
    Gp blocks are the stationary DoubleRow operand ([128, 2, 128] per
    (g, n-half)); 512 message rows stream per matmul into a [128, 2, 512]
    f32 PSUM pair holding parity transposed. g is the OUTER matmul loop
    so each arriving (c, g) piece feeds two matmuls (h = 0, 1).
  - The PE pstate ramps 1.2 -> 2.4 GHz only after ~9 us of wall-clock
    from the first PE op (measured 427 -> 216 ns per 512-stream matmul),
    so cheap 128-stream warmup matmuls on zeroed scratch start the ramp
    clock right after the instruction-load phase and idle-fill until the
    first loads land (~2.6 us DMA-completion semaphore latency after the
    data is actually in SBUF).
  - Raw bass instead of TileContext: hand-placed semaphores (one per
    load piece -- HWDGE queue shards complete out of order across
    transfers, so a single counting semaphore is unsafe), no Tile
    entry/exit barrier choreography (~1.5 us saved at the tail). Pool
    waits for the parity stores then range-clears the semaphores so a
    re-execution of the NEFF starts clean (same scheme Tile uses).
  - HWDGE "dynamic" queues generate descriptors ON the issuing sequencer
    (~0.6 us DIRECT2D per dma_start): loads ride the sync ring in
    consumption order (Gp g0 first), parity stores ride the scalar ring.
    ACT never computes (scalar.copy triggers a 1.3 us ACT_TABLE_LOAD
    inside the context-entry barrier, gating the first loads).
  - Mod 2 through integers (TS bitvec ops can't cast, AluOp.mod invalid
    ISA): DVE evicts PSUM f32 -> i16 (exact, sums <= 1000) and ANDs with
    1 at the 2x 16-bit rate, one whole-chunk [128, 2, 512] op pair per
    chunk; i16 parity stores per chunk; host upcasts i16 0/1 -> f32.
  - Host gathers: un-transpose parity and concatenate with the original
    f32 message bits.
"""

import contextlib
import os
import sys

import numpy as np

if os.path.isdir("/opt/trn_rl_repo") and "/opt/trn_rl_repo" not in sys.path:
    sys.path.insert(0, "/opt/trn_rl_repo")

import ml_dtypes

import concourse.bacc as bacc
import concourse.mybir as mybir
from concourse.bass_utils import run_bass_kernel_spmd

BATCH = 16384
MSG = 1000
NPAR = 256
NCORES = 8
ROWS = BATCH // NCORES  # 2048
P = 128
KB = 4  # k pair-blocks of 256; padded K = 1024
KPAD = KB * 2 * P
CH = 4 * P  # rows streamed per matmul (one PSUM bank of f32)

# test.py pokes these for profiling
TRACE = False
LAST_RESULT = None

_CACHE = {}

F8 = mybir.dt.float8e4
I16 = mybir.dt.int16
F32 = mybir.dt.float32


ALL_ENGINES = ("sync", "scalar", "vector", "tensor", "gpsimd")


def build_nc(rows=ROWS, warm_mms=34):
    n_chunks = rows // CH
    nc = bacc.Bacc("TRN2", target_bir_lowering=False, debug=False)
    msgt = nc.dram_tensor(
        "msgt", [P, n_chunks, KB, 2, CH], F8, kind="ExternalInput"
    )
    gp = nc.dram_tensor("gp", [P, KB, 2, NPAR], F8, kind="ExternalInput")
    out = nc.dram_tensor(
        "out", [P, n_chunks, 2, CH], I16, kind="ExternalOutput"
    )
    gp2 = gp[:, :, :, :].rearrange("q g i n -> q (g i n)")
    msgt2 = msgt[:, :, :, :, :].rearrange("q c g i m -> q (c g i m)")
    out2 = out[:, :, :, :].rearrange("q c h m -> q (c h m)")

    stack = contextlib.ExitStack()
    with stack:
        sem = {}

        def S(name):
            if name not in sem:
                sem[name] = stack.enter_context(nc.semaphore(f"s_{name}"))
            return sem[name]

        piece = 2 * CH
        sb = lambda name, shape, dt: stack.enter_context(
            nc.sbuf_tensor(name, shape, dt)
        )
        wW = sb("wW", [P, P], F8)
        gsb0 = sb("gsb0", [P, 2 * NPAR], F8)
        m0p = [sb(f"m0p{g}", [P, piece], F8) for g in range(KB)]
        gsb123 = sb("gsb123", [P, 3 * 2 * NPAR], F8)
        ctiles = {
            (c, g): sb(f"c{c}g{g}", [P, piece], F8)
            for c in range(1, n_chunks)
            for g in range(KB)
        }
        ci = {par: sb(f"ci{par}", [P, 2, CH], I16) for par in range(2)}
        ev = {c: sb(f"e{c}", [P, 2, CH], I16) for c in range(n_chunks)}

        pacc = {
            par: stack.enter_context(
                nc.psum_tensor(f"acc{par}", [P, 2, CH], F32)
            )
            for par in range(2)
        }
        wacc = stack.enter_context(nc.psum_tensor("wacc", [P, P], F32))

        # No entry clears/barrier: semaphores are zero on a fresh NEFF
        # execution (same assumption Tile makes -- it range-clears at the
        # END of the run for re-run hygiene; we do the same on Pool).

        # ---- loads split across BOTH HWDGE rings so descriptor
        # generation (~0.65 us per dma_start, serial per sequencer) runs
        # in parallel; each ring's FIFO is in consumption order and the
        # wire interleaves the two queues pairwise.
        sync_loads = [
            ("m0p0", m0p[0], msgt2[:, 0:piece]),
            ("m0p1", m0p[1], msgt2[:, piece : 2 * piece]),
            ("m0p2", m0p[2], msgt2[:, 2 * piece : 3 * piece]),
        ]
        scalar_loads = [
            ("gsb0", gsb0, gp2[:, 0 : 2 * NPAR]),
            ("gsb123", gsb123, gp2[:, 2 * NPAR :]),
            ("m0p3", m0p[3], msgt2[:, 3 * piece : 4 * piece]),
        ]
        for c in range(1, n_chunks):
            base = c * KB * piece
            for g in range(KB):
                ring = sync_loads if g % 2 == 0 else scalar_loads
                ring.append(
                    (
                        "c%dg%d" % (c, g),
                        ctiles[(c, g)],
                        msgt2[:, base + g * piece : base + (g + 1) * piece],
                    )
                )
        for name, dst, src in sync_loads:
            nc.sync.dma_start(out=dst[:, :], in_=src).then_inc(
                S("L" + name), 16
            )
        for name, dst, src in scalar_loads:
            nc.scalar.dma_start(out=dst[:, :], in_=src).then_inc(
                S("L" + name), 16
            )

        lc = n_chunks - 1  # last chunk evicts/stores per 512-row half

        # ---- DVE: memset warm weights, then per-chunk evict chain
        nc.vector.memset(wW[:, :], 0).then_inc(S("w"), 1)
        for c in range(n_chunks - 1):
            par = c % 2
            nc.vector.wait_ge(S(f"mm{c}"), 1)
            if c >= 2:
                # WAR on ci[par] (self-wait; appeases the race checker)
                nc.vector.wait_ge(S(f"and{c - 2}"), 1)
            nc.vector.tensor_copy(ci[par][:, :, :], pacc[par][:, :, :]).then_inc(
                S(f"ev{c}"), 1
            )
            # RAW on ci[par] within DVE (self-wait)
            nc.vector.wait_ge(S(f"ev{c}"), 1)
            nc.vector.tensor_scalar(
                ev[c][:, :, :],
                ci[par][:, :, :],
                1,
                None,
                mybir.AluOpType.bitwise_and,
            ).then_inc(S(f"and{c}"), 1)
        parl = lc % 2
        for h in range(2):
            nc.vector.wait_ge(S(f"mm{lc}h{h}"), 1)
            if lc >= 2 and h == 0:
                nc.vector.wait_ge(S(f"and{lc - 2}"), 1)
            nc.vector.tensor_copy(
                ci[parl][:, h, :], pacc[parl][:, h, :]
            ).then_inc(S(f"ev{lc}h{h}"), 1)
            nc.vector.wait_ge(S(f"ev{lc}h{h}"), 1)
            nc.vector.tensor_scalar(
                ev[lc][:, h, :],
                ci[parl][:, h, :],
                1,
                None,
                mybir.AluOpType.bitwise_and,
            ).then_inc(S(f"and{lc}h{h}"), 1)

        # ---- stores on scalar ring (behind its loads)
        n_stores = 0
        for c in range(n_chunks - 1):
            nc.scalar.wait_ge(S(f"and{c}"), 1)
            nc.scalar.dma_start(
                out=out2[:, c * 2 * CH : (c + 1) * 2 * CH],
                in_=ev[c][:, :, :].rearrange("q h m -> q (h m)"),
            ).then_inc(S("st"), 16)
            n_stores += 1
        for h in range(2):
            nc.scalar.wait_ge(S(f"and{lc}h{h}"), 1)
            nc.scalar.dma_start(
                out=out2[:, (2 * lc + h) * CH : (2 * lc + h + 1) * CH],
                in_=ev[lc][:, h, :],
            ).then_inc(S("st"), 16)
            n_stores += 1

        # ---- PE: warmup then the real matmuls
        def gsbv(g):
            if g == 0:
                return gsb0[:, :].rearrange("q (i n) -> q i n", n=NPAR)
            s = gsb123[:, (g - 1) * 2 * NPAR : g * 2 * NPAR]
            return s.rearrange("q (i n) -> q i n", n=NPAR)

        def mtv(c, g):
            if c == 0:
                s = m0p[g][:, :]
            else:
                s = ctiles[(c, g)][:, :]
            return s.rearrange("q (i m) -> q i m", m=CH)

        # load-sem name each (c, g) moving operand arrives under
        def ld_of(c, g):
            if c == 0:
                return f"Lm0p{g}"
            return "Lc%dg%d" % (c, g)

        nc.tensor.wait_ge(S("w"), 1)
        for _ in range(warm_mms):
            nc.tensor.matmul(
                wacc[:, :], wW[:, :], wW[:, :], start=True, stop=True
            )

        waited = set()

        def pe_wait(name):
            if name not in waited:
                waited.add(name)
                nc.tensor.wait_ge(S(name), 16)

        for c in range(n_chunks):
            par = c % 2
            if c >= 2:
                # psum reuse: wait for chunk c-2's eviction
                nc.tensor.wait_ge(S(f"ev{c - 2}"), 1)
            # g OUTER: each newly arrived (c, g) piece feeds TWO matmuls
            # (h=0, h=1) before the next piece is needed, halving the
            # load-arrival rate the PE depends on
            for g in range(KB):
                pe_wait("Lgsb0" if g == 0 else "Lgsb123")
                pe_wait(ld_of(c, g))
                for h in range(2):
                    mm = nc.tensor.matmul(
                        pacc[par][:, h, :],
                        gsbv(g)[:, :, h * P : (h + 1) * P],
                        mtv(c, g)[:, :, :],
                        start=(g == 0),
                        stop=(g == KB - 1),
                        perf_mode=mybir.MatmulPerfMode.DoubleRow,
                    )
                    if c == lc and g == KB - 1:
                        # per-half completion for the split last-chunk evict
                        mm.then_inc(S(f"mm{lc}h{h}"), 1)
            if c < lc:
                mm.then_inc(S(f"mm{c}"), 1)

        # ---- make sure stores land before the NEFF retires, then zero
        # the semaphores (after a sem-only barrier) so a re-execution of
        # this NEFF starts clean
        nc.gpsimd.wait_ge(S("st"), 16 * n_stores)
        nc.all_engine_barrier(sem_only=True)
        nums = [h.num for h in sem.values()]
        nc.gpsimd.sem_clear(range(min(nums), max(nums) + 1))

    nc.compile()
    return nc


def prep_gp(Gp):
    """Pad Gp to 1024 rows and swizzle to [128, 4, 2, 256] fp8:
    gsw[q, g, i, n] = Gp_pad[256*g + 128*i + q, n]
    """
    gp = np.asarray(Gp, dtype=np.float32)
    gp_pad = np.zeros((KPAD, NPAR), dtype=np.float32)
    gp_pad[:MSG] = gp
    gsw = gp_pad.reshape(KB, 2, P, NPAR).transpose(2, 0, 1, 3)
    return np.ascontiguousarray(gsw).astype(ml_dtypes.float8_e4m3)


def prep_msgt(msg, rows=ROWS):
    """Cast 0/1 f32 message bits to fp8 (exact), pad k to 1024, and swizzle
    each `rows`-row slice to the transposed moving layout
    msgt[q, c, g, i, m] = msg[slice_row0 + 512c + m, 256g + 128i + q]."""
    f8 = np.zeros((msg.shape[0], KPAD), dtype=ml_dtypes.float8_e4m3)
    f8[:, :MSG] = msg.astype(ml_dtypes.float8_e4m3)
    n_chunks = rows // CH
    per_core = []
    for i in range(msg.shape[0] // rows):
        sl = f8[i * rows : (i + 1) * rows]
        # [c, m, g, i, q] -> [q, c, g, i, m]
        sw = sl.reshape(n_chunks, CH, KB, 2, P).transpose(4, 0, 2, 3, 1)
        per_core.append(np.ascontiguousarray(sw))
    return per_core


def parity_from_out(out_i16):
    """Device 'out' [128, n_chunks, 2, CH] i16 -> [rows, 256] f32."""
    o = np.asarray(out_i16)
    n_chunks = o.shape[1]
    # [nh, c, h, m] -> [c, m, h, nh] -> [rows, 256]
    return (
        o.transpose(1, 3, 2, 0)
        .reshape(n_chunks * CH, NPAR)
        .astype(np.float32)
    )


def kernel(message_bits, Gp):
    global LAST_RESULT
    msg = np.ascontiguousarray(np.asarray(message_bits, dtype=np.float32))
    assert msg.shape == (BATCH, MSG), msg.shape
    gsw = prep_gp(Gp)
    msg_cores = prep_msgt(msg)

    if "nc" not in _CACHE:
        _CACHE["nc"] = build_nc()
    nc = _CACHE["nc"]

    in_maps = [{"msgt": msg_cores[i], "gp": gsw} for i in range(NCORES)]
    res = run_bass_kernel_spmd(
        nc, in_maps, core_ids=list(range(NCORES)), trace=TRACE
    )
    LAST_RESULT = res

    full = np.empty((BATCH, MSG + NPAR), dtype=np.float32)
    full[:, :MSG] = msg
    for i, r in enumerate(res.results):
        full[i * ROWS : (i + 1) * ROWS, MSG:] = parity_from_out(r["out"])
    return full
